# revision 28
# baseline (speedup 1.0000x reference)
"""Trainium2 Bass kernel for nn_CoucheinitialeGNN (GNN edge-MLP + segment-sum normalize).

Math (reference):
    bucket = clip(int(dist), 0, 9); one_hot [E,10]
    h      = relu(dist @ W1 + b1)          [E,128]
    mlp    = relu(h @ W2 + b2)             [E,54]
    w      = concat([one_hot, mlp])        [E,64]
    d      = segment_sum(w, src, N)        [N,64]
    out    = w / d[src]   (0/0 := 0)       [E,64]

Strategy (node-major): shard nodes across 8 cores; per core sort nodes by
degree and pack 128 nodes per bin (partition = node, free axis = that
node's edges padded to the bin max degree D). The segment sum is then a
within-partition tree-reduce on the vector engine and the d[src] gather is
a free broadcast — no selection-matrix matmuls at all. The tensor engine
only computes the edge MLP: L1 as a K=9 bf16 matmul (hi/mid/lo split of
dist and W1, ~f32-exact) into an f32 hT, L2 as one fp32 [F,128]x[F,54]
matmul per 128-slot group. PRE-relu precision must track the reference's
f32 math closely: where a node's segment sum is dominated by one edge the
reference emits w/w = 1.0 for arbitrarily tiny w, so a sign flip of the
pre-relu value is a full-scale error. POST-relu everything is relative, so
w, r and the output are bf16 (|out| <= 1, tol 2e-2), halving store
traffic; the host converts to f32 and scatters rows back. Dead/always-
linear relu features are folded (and mostly promoted back) on the host.
"""

import numpy as np
import ml_dtypes

import concourse.bass as bass
import concourse.bacc as bacc
import concourse.tile as tile
import concourse.mybir as mybir
from concourse.bass_utils import run_bass_kernel_spmd

F32 = mybir.dt.float32
BF16 = mybir.dt.bfloat16

N_NODES = 100000
N_EDGES = 1600000
N_CORES = 8
THRESHOLD = 10.0

NPC = N_NODES // N_CORES                     # 12500 nodes per core
NBIN = -(-NPC // 128)                        # 98 bins of 128 nodes
EPS = 2e-38                                  # keeps 1/(d+eps) finite + normal

# L2 slot-group size: matmuls per PSUM tile / relu batch (9*54*4B < 2KB bank)
PM_JN = 9


# ---------------------------------------------------------------------------
# host-side weight folding (bitwise-identical to the proven baseline fold)
# ---------------------------------------------------------------------------

def fold_weights(W1, b1, W2, b2):
    """Split relu features into kinky (computed on device) and linear/dead
    (folded into two extra contraction rows: dist-coef and const).  Linear
    features are promoted back into the device-computed set while room
    remains so knife-edge relu channels see the exact f32 math."""
    W1 = np.asarray(W1, np.float32).reshape(-1)       # [128]
    b1 = np.asarray(b1, np.float32).reshape(-1)       # [128]
    W2 = np.asarray(W2, np.float32)                   # [128, 54]
    b2 = np.asarray(b2, np.float32).reshape(-1)       # [54]
    H, O = W2.shape

    lo = b1                                            # value at d -> 0+
    hi = THRESHOLD * W1 + b1                           # value at d = 10
    with np.errstate(divide="ignore", invalid="ignore"):
        t = np.where(W1 != 0, -b1 / W1, np.inf)
    kinky = (t > -0.5) & (t < THRESHOLD + 0.5) & (W1 != 0)
    dead = ~kinky & (np.maximum(lo, hi) <= 0)
    linear = ~kinky & ~dead                            # relu == identity on (0,10]

    room = 128 - 2 - int(kinky.sum())
    lin_idx = np.nonzero(linear)[0]
    if len(lin_idx) > room:
        impact = np.maximum(np.abs(lo), np.abs(hi))[lin_idx] * \
            np.abs(W2[lin_idx]).max(1)
        lin_idx = lin_idx[np.argsort(-impact)[:room]]
    promote = np.zeros_like(linear)
    promote[lin_idx] = True
    kinky = kinky | promote
    linear = linear & ~promote

    KH = int(kinky.sum())
    assert KH + 2 <= 128, f"kinky feature count {KH} too large"

    A = (W2[linear].astype(np.float64) * W1[linear, None].astype(np.float64)).sum(0)
    C = (W2[linear].astype(np.float64) * b1[linear, None].astype(np.float64)).sum(0) \
        + b2.astype(np.float64)

    # layer-1 lhsT [9, KH+2] bf16, paired with rhs rows
    # [dh, dh, dh, dm, dm, dl, v, v, v]:
    #   col j<KH: [wh, wm, wl, wh, wm, wh, b1h, b1m, b1l]
    #   col KH:   dist-copy [1,0,0,1,0,1,0,0,0] -> dh+dm+dl = dist
    #   col KH+1: ones-copy [0,...,0,1,0,0]     -> v (1 real / 0 pad)
    def split3(v):
        hi_ = v.astype(ml_dtypes.bfloat16)
        r = v - hi_.astype(np.float32)
        mid = r.astype(ml_dtypes.bfloat16)
        lo_ = (r - mid.astype(np.float32)).astype(ml_dtypes.bfloat16)
        return hi_, mid, lo_

    W1k = W1[kinky]
    b1k = b1[kinky]
    wh, wm, wl = split3(W1k)
    bh, bm, bl = split3(b1k)
    F = KH + 2
    l1 = np.zeros((9, F), ml_dtypes.bfloat16)
    for i, row in enumerate([wh, wm, wl, wh, wm, wh, bh, bm, bl]):
        l1[i, :KH] = row
    l1[0, KH] = 1.0
    l1[3, KH] = 1.0
    l1[5, KH] = 1.0
    l1[6, KH + 1] = 1.0

    w2aug = np.zeros((F, O), np.float32)
    w2aug[:KH] = W2[kinky]
    w2aug[KH] = A.astype(np.float32)
    w2aug[KH + 1] = C.astype(np.float32)
    # bf16 hi/lo pair of the L2 weights (used with the bf16 hi/lo pair of h:
    # x ~= hh*w2h + hh*w2l + hl*w2h, dropping hl*w2l <= 2^-18 |h||w2|)
    w2h = w2aug.astype(ml_dtypes.bfloat16)
    w2l = (w2aug - w2h.astype(np.float32)).astype(ml_dtypes.bfloat16)
    return l1, w2h, w2l, KH


# ---------------------------------------------------------------------------
# host-side edge partitioning (node-major bins)
# ---------------------------------------------------------------------------

def plan(src):
    """Sort edges by src, shard nodes across cores, sort nodes by degree and
    pack 128 per bin.  Returns per-core edge->slot data and the shared
    per-bin padded degree profile D (even, identical across cores)."""
    order = np.argsort(src, kind="stable")
    ssrc = src[order]
    bounds = np.searchsorted(ssrc, np.arange(N_CORES + 1) * NPC)
    cores = []
    Dmat = np.zeros((N_CORES, NBIN), np.int64)
    for k in range(N_CORES):
        lo, hi = bounds[k], bounds[k + 1]
        eids = order[lo:hi]
        lsrc = (ssrc[lo:hi] - k * NPC).astype(np.int64)
        deg = np.bincount(lsrc, minlength=NPC)
        nodeord = np.argsort(-deg, kind="stable")
        rank = np.empty(NPC, np.int64)
        rank[nodeord] = np.arange(NPC)
        degs = deg[nodeord]
        dpad = np.zeros(NBIN * 128, np.int64)
        dpad[:NPC] = degs
        Dmat[k] = dpad.reshape(NBIN, 128).max(1)
        starts = np.concatenate([[0], np.cumsum(deg)])
        j = np.arange(len(lsrc)) - starts[lsrc]
        cores.append({"eids": eids, "lsrc": lsrc, "rank": rank, "j": j})
    Dm = Dmat.max(0)
    D = Dm + (Dm & 1)                        # even so halving trees stay simple
    cbase = np.concatenate([[0], np.cumsum(128 * D)])
    dbase = np.concatenate([[0], np.cumsum(D)])
    return cores, D, cbase, dbase, int(cbase[-1]), int(dbase[-1])


def prepare(cores, D, cbase, dbase, EP, DSUM, dist):
    in_maps = []
    gids_all = []
    for c in cores:
        eids, lsrc, j = c["eids"], c["lsrc"], c["j"]
        r = c["rank"][lsrc]
        p = r % 128
        b = r // 128
        col = cbase[b] + j * 128 + p         # L1 rhs column of this edge
        row = cbase[b] + p * D[b] + j        # output DRAM row of this edge

        de = dist[eids]
        distv = np.zeros(EP, np.float32)
        distv[col] = de
        valid = np.zeros(EP, np.float32)
        valid[col] = 1.0
        dh = distv.astype(ml_dtypes.bfloat16)
        r1 = distv - dh.astype(np.float32)
        dm = r1.astype(ml_dtypes.bfloat16)
        dl = (r1 - dm.astype(np.float32)).astype(ml_dtypes.bfloat16)
        rhs9 = np.empty((9, EP), ml_dtypes.bfloat16)
        rhs9[0] = dh
        rhs9[1] = dh
        rhs9[2] = dh
        rhs9[3] = dm
        rhs9[4] = dm
        rhs9[5] = dl
        rhs9[6] = valid
        rhs9[7] = valid
        rhs9[8] = valid

        bucketf = np.full((128, DSUM), -1.0, ml_dtypes.bfloat16)
        bucketf[p, dbase[b] + j] = np.clip(de.astype(np.int32), 0, 9)

        gids = np.full(EP, -1, np.int64)
        gids[row] = eids
        in_maps.append({"rhs9": rhs9, "bucketf": bucketf})
        gids_all.append(gids)
    return in_maps, gids_all


# ---------------------------------------------------------------------------
# device kernel
# ---------------------------------------------------------------------------

_NC_CACHE = {}


def build_kernel(F, l1_np, w2h_np, w2l_np, D, cbase, dbase, EP, DSUM):
    nc = bacc.Bacc("TRN2", target_bir_lowering=False, debug=False, num_devices=N_CORES)

    rhs9 = nc.dram_tensor("rhs9", [9, EP], BF16, kind="ExternalInput")
    bucketf = nc.dram_tensor("bucketf", [128, DSUM], BF16, kind="ExternalInput")
    out = nc.dram_tensor("out", [EP, 64], BF16, kind="ExternalOutput")

    l1_t = nc.inline_tensor(np.ascontiguousarray(l1_np), name="l1w")
    w2h_t = nc.inline_tensor(np.ascontiguousarray(w2h_np), name="w2h")
    w2l_t = nc.inline_tensor(np.ascontiguousarray(w2l_np), name="w2l")
    iota10_np = np.broadcast_to(
        np.arange(10).astype(ml_dtypes.bfloat16), (128, 10)
    ).copy()
    i10_t = nc.inline_tensor(iota10_np, name="iota10")

    Relu = mybir.ActivationFunctionType.Relu
    ADD = mybir.AluOpType.add
    MULT = mybir.AluOpType.mult
    ISEQ = mybir.AluOpType.is_equal

    with tile.TileContext(nc) as tc, nc.allow_low_precision(
        reason="bf16 partial sums / outputs are within the 2e-2 tolerance"
    ):
        with (
            tc.tile_pool(name="const", bufs=1) as cpool,
            tc.tile_pool(name="io", bufs=4) as iopool,
            tc.tile_pool(name="work", bufs=4) as wpool,
            tc.tile_pool(name="ps1", bufs=3, space="PSUM") as ps1p,
            tc.tile_pool(name="psm", bufs=4, space="PSUM") as psmp,
        ):
            l1c = cpool.tile([9, F], BF16)
            w2hc = cpool.tile([F, 54], BF16)
            w2lc = cpool.tile([F, 54], BF16)
            i10 = cpool.tile([128, 10], BF16)
            bkall = cpool.tile([128, DSUM], BF16)
            nc.sync.dma_start(l1c[:], l1_t[:, :])
            nc.sync.dma_start(w2hc[:], w2h_t[:, :])
            nc.sync.dma_start(w2lc[:], w2l_t[:, :])
            nc.sync.dma_start(i10[:], i10_t[:, :])
            nc.sync.dma_start(bkall[:], bucketf[:, :])

            for b in range(NBIN):
                Db = int(D[b])
                if Db == 0:
                    continue
                S = 128 * Db
                e0 = int(cbase[b])
                d0 = int(dbase[b])

                t_rhs = iopool.tile([9, S], BF16, tag="rhs")
                nc.sync.dma_start(t_rhs[:], rhs9[:, e0:e0 + S])

                # ---- L1: h = relu(l1^T @ rhs9) split into a bf16 hi/lo pair
                # (hh on scalar from PSUM, hl on vector as (p1 max 0) - hh)
                hh = wpool.tile([F, S], BF16, tag="hh")
                hl = wpool.tile([F, S], BF16, tag="hl")
                c0 = 0
                while c0 < S:
                    cw = min(512, S - c0)
                    p1 = ps1p.tile([F, 512], F32, tag="p1")
                    nc.tensor.matmul(
                        out=p1[:, :cw], lhsT=l1c[:], rhs=t_rhs[:, c0:c0 + cw],
                        start=True, stop=True,
                    )
                    nc.scalar.activation(hh[:, c0:c0 + cw], p1[:, :cw], Relu)
                    nc.vector.scalar_tensor_tensor(
                        out=hl[:, c0:c0 + cw], in0=p1[:, :cw], scalar=0.0,
                        in1=hh[:, c0:c0 + cw],
                        op0=mybir.AluOpType.max, op1=mybir.AluOpType.subtract,
                    )
                    c0 += cw

                # ---- w [128, Db, 64]: one-hot(bucket) | relu(L2)
                w_t = wpool.tile([128, Db, 64], BF16, tag="w")
                nc.vector.tensor_tensor(
                    out=w_t[:, :, 0:10],
                    in0=bkall[:, d0:d0 + Db].unsqueeze(2).to_broadcast([128, Db, 10]),
                    in1=i10[:].unsqueeze(1).to_broadcast([128, Db, 10]),
                    op=ISEQ,
                )
                for j0 in range(0, Db, PM_JN):
                    jn = min(PM_JN, Db - j0)
                    pm = psmp.tile([128, PM_JN * 54], F32, tag="pm")
                    for jj in range(jn):
                        jx = j0 + jj
                        sl = slice(jx * 128, (jx + 1) * 128)
                        o = slice(jj * 54, (jj + 1) * 54)
                        nc.tensor.matmul(
                            out=pm[:, o], lhsT=hh[:, sl], rhs=w2hc[:],
                            start=True, stop=False,
                        )
                        nc.tensor.matmul(
                            out=pm[:, o], lhsT=hh[:, sl], rhs=w2lc[:],
                            start=False, stop=False,
                        )
                        nc.tensor.matmul(
                            out=pm[:, o], lhsT=hl[:, sl], rhs=w2hc[:],
                            start=False, stop=True,
                        )
                    nc.scalar.activation(
                        w_t[:, j0:j0 + jn, 10:64], pm[:, :jn * 54], Relu,
                    )

                # ---- d+eps = eps + sum_j w[:, j, :] (within-partition tree;
                # eps fused into the final add via scalar_tensor_tensor)
                dsb = wpool.tile([128, 64], F32, tag="dsb")

                def final_add(a_ap, b_ap):
                    nc.vector.scalar_tensor_tensor(
                        out=dsb[:].unsqueeze(1), in0=a_ap, scalar=EPS,
                        in1=b_ap, op0=ADD, op1=ADD,
                    )

                if Db == 2:
                    final_add(w_t[:, 0:1, :], w_t[:, 1:2, :])
                else:
                    red = wpool.tile([128, Db // 2, 64], BF16, tag="red")
                    srcv = w_t
                    cur = Db
                    while cur > 2:
                        half = cur // 2
                        nc.vector.tensor_tensor(
                            out=red[:, 0:half, :],
                            in0=srcv[:, 0:half, :],
                            in1=srcv[:, half:2 * half, :],
                            op=ADD,
                        )
                        if cur & 1:
                            nc.vector.tensor_tensor(
                                out=red[:, 0:1, :],
                                in0=red[:, 0:1, :],
                                in1=srcv[:, 2 * half:cur, :],
                                op=ADD,
                            )
                        srcv = red
                        cur = half
                    if cur == 2:
                        final_add(srcv[:, 0:1, :], srcv[:, 1:2, :])
                    else:
                        nc.vector.tensor_scalar_add(
                            dsb[:].unsqueeze(1), srcv[:, 0:1, :], EPS,
                        )

                # ---- r = 1/(d+eps) f32 (mult below is 1x either way)
                rblk = wpool.tile([128, 64], F32, tag="rblk")
                nc.vector.reciprocal_approx_fast(out=rblk[:], in_=dsb[:])

                # ---- out = w * r (broadcast over slots), bf16 store
                out_t = wpool.tile([128, Db, 64], BF16, tag="ot")
                nc.vector.tensor_tensor(
                    out=out_t[:],
                    in0=w_t[:],
                    in1=rblk[:].unsqueeze(1).to_broadcast([128, Db, 64]),
                    op=MULT,
                )
                nc.sync.dma_start(out[e0:e0 + S, :], out_t[:])
    nc.compile()
    return nc


# ---------------------------------------------------------------------------
# entry point
# ---------------------------------------------------------------------------

def kernel(x, edge_index, edge_attr, W1, b1, W2, b2):
    src = np.asarray(edge_index)[0].astype(np.int64)
    dist = np.asarray(edge_attr, np.float32)[:, 0]

    l1_np, w2h_np, w2l_np, KH = fold_weights(W1, b1, W2, b2)
    F = KH + 2
    cores, D, cbase, dbase, EP, DSUM = plan(src)

    key = (F, D.tobytes(), l1_np.tobytes(), w2h_np.tobytes())
    nc = _NC_CACHE.get(key)
    if nc is None:
        nc = build_kernel(F, l1_np, w2h_np, w2l_np, D, cbase, dbase, EP, DSUM)
        _NC_CACHE[key] = nc

    in_maps, gids_all = prepare(cores, D, cbase, dbase, EP, DSUM, dist)
    res = run_bass_kernel_spmd(nc, in_maps, core_ids=list(range(N_CORES)))

    final = np.empty((N_EDGES, 64), np.float32)
    for k in range(N_CORES):
        o = np.asarray(res.results[k]["out"]).astype(np.float32)
        gids = gids_all[k]
        m = gids >= 0
        final[gids[m]] = o[m]
    return final
